# revision 12
# baseline (speedup 1.0000x reference)
"""MoE router kernel for Trainium2 (8 NeuronCores, data-parallel over tokens).

Computes, for x:[32768,2048] f32 and gate_w:[2048,64] f32:
  logits = x @ gate_w
  top2 values/indices, top2 softmax weights
  probs = softmax(logits); load_balance_loss = E * sum(f * P)

Sharding: token dim split 8 ways (4096 tokens/core); gate replicated.
tokens_per_expert is computed on the host from the returned indices;
per-core softmax-prob column sums are reduced on host for the loss.

The fp32 matmul runs at ~fp32 precision via a bf16 hi/lo split:
  x = xh + xl, w = wh + wl, logits = xh@wh + xh@wl + xl@wh
as three bf16 matmuls per 128-hidden chunk accumulated in one PSUM tile.

Host prepares x in exact DMA consumption order
  [block, partition, hi/lo, chunk, token]
so each input block transfer is one dense 2D copy (16KB contiguous per
partition -> 4KB DMA packets at full HBM rate). Tokens are permuted so
mm-tile j / partition p holds local token p*32+j, making the [4096,2]
outputs contiguous staged DMAs.

Self-contained: hardcodes shapes; imports only concourse (on PYTHONPATH).
"""

import numpy as np

N_TOKENS = 32768
HIDDEN = 2048
NUM_EXPERTS = 64
TOP_K = 2

N_CORES = 8
NP = N_TOKENS // N_CORES  # tokens per core = 4096
KCH = HIDDEN // 128       # hidden chunks of 128 = 16
BLK = 256                 # tokens per DMA block
NBLK = NP // BLK          # 8
TPB = BLK // 128          # matmul tiles per block = 4
NTILE = NP // 128         # 32 mm-tiles per core

_CACHE = {}


def _build_nc():
    import concourse.bacc as bacc
    import concourse.tile as tile
    from concourse import mybir

    f32 = mybir.dt.float32
    bf16 = mybir.dt.bfloat16
    u32 = mybir.dt.uint32
    Alu = mybir.AluOpType
    Act = mybir.ActivationFunctionType

    nc = bacc.Bacc("TRN2", target_bir_lowering=False, debug=False)

    xc = nc.dram_tensor("xc", [NBLK, 128, 2, KCH, BLK], bf16,
                        kind="ExternalInput").ap()
    whl = nc.dram_tensor("whl", [HIDDEN, 2 * NUM_EXPERTS], bf16,
                         kind="ExternalInput").ap()
    w_out = nc.dram_tensor("w_out", [NP, TOP_K], f32, kind="ExternalOutput").ap()
    i_out = nc.dram_tensor("i_out", [NP, TOP_K], u32, kind="ExternalOutput").ap()
    p_out = nc.dram_tensor("p_out", [1, NUM_EXPERTS], f32, kind="ExternalOutput").ap()

    wv = w_out.rearrange("(p j) k -> p j k", p=128)
    iv = i_out.rearrange("(p j) k -> p j k", p=128)

    with tile.TileContext(nc) as tc:
        with tc.tile_pool(name="xpool", bufs=6) as xpool, \
             tc.tile_pool(name="cpool", bufs=1) as cpool, \
             tc.tile_pool(name="lpool", bufs=4) as lpool, \
             tc.tile_pool(name="spool", bufs=6) as spool, \
             tc.tile_pool(name="pspool", bufs=7, space="PSUM") as pspool, \
             tc.tile_pool(name="pfpool", bufs=1, space="PSUM") as pfpool:

            # ---- constants (HWDGE sync ring, issued before the blocks) ----
            w_sb = cpool.tile([128, KCH, 2 * NUM_EXPERTS], bf16)
            nc.sync.dma_start(out=w_sb,
                              in_=whl.rearrange("(K p) e -> p K e", p=128))

            accP = cpool.tile([128, NUM_EXPERTS], f32)
            nc.vector.memset(accP, 0.0)

            ones = cpool.tile([128, 1], f32)
            nc.vector.memset(ones, 1.0)

            # output staging, one tile per quarter so each stores early
            NQ = 4
            QT = NTILE // NQ
            wstage = [cpool.tile([128, QT, TOP_K], f32,
                                 name=f"wstage{h}", tag=f"wstage{h}")
                      for h in range(NQ)]
            istage = [cpool.tile([128, QT, TOP_K], u32,
                                 name=f"istage{h}", tag=f"istage{h}")
                      for h in range(NQ)]

            for b in range(NBLK):
                xblk = xpool.tile([128, 2, KCH, BLK], bf16, tag="xblk")
                if b == 0:
                    # split the first block across both HWDGE rings so the
                    # pipeline starts sooner
                    h0 = BLK // 2
                    nc.sync.dma_start(out=xblk[:, :, :, 0:h0],
                                      in_=xc[0][:, :, :, 0:h0])
                    nc.scalar.dma_start(out=xblk[:, :, :, h0:BLK],
                                        in_=xc[0][:, :, :, h0:BLK])
                else:
                    nc.sync.dma_start(out=xblk, in_=xc[b])

                for jj in range(TPB):
                    j = b * TPB + jj
                    half, jh = divmod(j, QT)
                    ts = jj * 128
                    # ps[:, :64] = xh@wh + xl@wh ; ps[:, 64:] = xh@wl
                    ps = pspool.tile([128, 2 * NUM_EXPERTS], f32, tag="ps")
                    for k in range(KCH):
                        nc.tensor.matmul(ps, lhsT=xblk[:, 0, k, ts:ts + 128],
                                         rhs=w_sb[:, k, :],
                                         start=(k == 0), stop=False)
                        nc.tensor.matmul(ps[:, 0:NUM_EXPERTS],
                                         lhsT=xblk[:, 1, k, ts:ts + 128],
                                         rhs=w_sb[:, k, 0:NUM_EXPERTS],
                                         start=False, stop=(k == KCH - 1))

                    hi = lpool.tile([128, NUM_EXPERTS], f32, tag="hi")
                    nc.scalar.activation(hi, ps[:, NUM_EXPERTS:2 * NUM_EXPERTS],
                                         Act.Copy)
                    logit = lpool.tile([128, NUM_EXPERTS], f32, tag="logit")
                    nc.vector.tensor_add(logit, ps[:, 0:NUM_EXPERTS], hi)

                    mx = spool.tile([128, 8], f32, tag="mx")
                    nc.vector.max(out=mx, in_=logit)
                    mi = spool.tile([128, 8], u32, tag="mi")
                    nc.vector.max_index(out=mi, in_max=mx, in_values=logit)

                    negm = spool.tile([128, 1], f32, tag="negm")
                    nc.scalar.activation(negm, mx[:, 0:1], Act.Copy, bias=0.0,
                                         scale=-1.0)

                    # full-row softmax numerator + row sum (for probs partials)
                    ex = lpool.tile([128, NUM_EXPERTS], f32, tag="ex")
                    s = spool.tile([128, 1], f32, tag="s")
                    nc.scalar.activation(ex, logit, Act.Exp, bias=negm, scale=1.0,
                                         accum_out=s)

                    # top-2 softmax: [1, exp(m2-m1)] / (1 + exp(m2-m1))
                    pairex = spool.tile([128, TOP_K], f32, tag="pairex")
                    denom = spool.tile([128, 1], f32, tag="denom")
                    nc.scalar.activation(pairex, mx[:, 0:TOP_K], Act.Exp, bias=negm,
                                         scale=1.0, accum_out=denom)
                    rden = spool.tile([128, 1], f32, tag="rden")
                    nc.vector.reciprocal(rden, denom)
                    nc.vector.tensor_scalar_mul(wstage[half][:, jh, :], pairex, rden)

                    rs = spool.tile([128, 1], f32, tag="rs")
                    nc.vector.reciprocal(rs, s)
                    # accP += ex * (1/s)
                    nc.vector.scalar_tensor_tensor(
                        out=accP, in0=ex, scalar=rs, in1=accP,
                        op0=Alu.mult, op1=Alu.add)

                    nc.vector.tensor_copy(istage[half][:, jh, :], mi[:, 0:TOP_K])

                # store each completed quarter on the scalar (ACT) HWDGE ring
                done = b * TPB + TPB
                if done % QT == 0:
                    half = done // QT - 1
                    j0 = half * QT
                    nc.scalar.dma_start(out=wv[:, j0:j0 + QT, :],
                                        in_=wstage[half])
                    nc.scalar.dma_start(out=iv[:, j0:j0 + QT, :],
                                        in_=istage[half])

            # column-sum accP over the 128 partitions: [1, 64] = ones.T @ accP
            pps = pfpool.tile([1, NUM_EXPERTS], f32, tag="pps")
            nc.tensor.matmul(pps, lhsT=ones, rhs=accP, start=True, stop=True)
            psb = cpool.tile([1, NUM_EXPERTS], f32)
            nc.scalar.activation(psb, pps, Act.Copy)
            nc.scalar.dma_start(out=p_out, in_=psb)

    nc.compile()
    return nc


def _get_nc():
    if "nc" not in _CACHE:
        _CACHE["nc"] = _build_nc()
    return _CACHE["nc"]


def _prep_core(x_shard):
    """x_shard [NP, HIDDEN] f32 -> [NBLK, 128, 2, KCH, BLK] bf16.

    Element [b, p, hl, k, t] = hi/lo part of x_shard[tok, k*128+p] for
    tok = perm(b*BLK+t) where kernel column j*128+q holds local token
    q*32+j (so mm-tile j / partition q gets token q*32+j).
    """
    import ml_dtypes

    bf = ml_dtypes.bfloat16
    xt = np.ascontiguousarray(x_shard.T)                # [H, NP] cols = tokens
    xt = xt.reshape(HIDDEN, 128, NTILE).transpose(0, 2, 1).reshape(HIDDEN, NP)
    xh = xt.astype(bf)
    xl = (xt - xh.astype(np.float32)).astype(bf)
    a = np.stack([xh, xl], axis=0)                      # [2, H, NP]
    a = a.reshape(2, KCH, 128, NBLK, BLK)
    a = a.transpose(3, 2, 0, 1, 4)                      # [NBLK, 128, 2, KCH, BLK]
    return np.ascontiguousarray(a)


def _prep_w(gate_w):
    import ml_dtypes

    bf = ml_dtypes.bfloat16
    wh = gate_w.astype(bf)
    wl = (gate_w - wh.astype(np.float32)).astype(bf)
    return np.ascontiguousarray(np.concatenate([wh, wl], axis=1))


def _run(x, gate_w, trace=False, trace_cores=None):
    from concourse import bass_utils

    nc = _get_nc()
    x = np.asarray(x, dtype=np.float32)
    gate_w = np.asarray(gate_w, dtype=np.float32)

    whl = _prep_w(gate_w)
    in_maps = [
        {"xc": _prep_core(x[c * NP:(c + 1) * NP]), "whl": whl}
        for c in range(N_CORES)
    ]
    kw = {}
    if trace:
        kw = {"trace": True,
              "trace_cores": trace_cores if trace_cores is not None else [0]}
    res = bass_utils.run_bass_kernel_spmd(nc, in_maps, list(range(N_CORES)), **kw)
    outs = res.results

    w = np.concatenate([outs[c]["w_out"] for c in range(N_CORES)], axis=0)
    idx = np.concatenate([outs[c]["i_out"] for c in range(N_CORES)],
                         axis=0).astype(np.int32)
    psum = np.stack([outs[c]["p_out"][0] for c in range(N_CORES)],
                    axis=0).sum(axis=0, dtype=np.float64)
    counts = np.bincount(idx.reshape(-1), minlength=NUM_EXPERTS).astype(np.float64)
    f = counts / float(N_TOKENS)
    P = psum / float(N_TOKENS)
    loss = np.float32(NUM_EXPERTS * np.sum(f * P))
    return (w.astype(np.float32), idx, loss), res


def kernel(x, gate_w):
    (w, idx, loss), _ = _run(x, gate_w)
    return w, idx, loss


# revision 13
# speedup vs baseline: 1.0615x; 1.0615x over previous
"""MoE router kernel for Trainium2 (8 NeuronCores, data-parallel over tokens).

Computes, for x:[32768,2048] f32 and gate_w:[2048,64] f32:
  logits = x @ gate_w
  top2 values/indices, top2 softmax weights
  probs = softmax(logits); load_balance_loss = E * sum(f * P)

Sharding: token dim split 8 ways (4096 tokens/core); gate replicated.
tokens_per_expert is computed on the host from the returned indices;
per-core softmax-prob column sums are reduced on host for the loss.

The fp32 matmul runs at ~fp32 precision via a bf16 hi/lo split:
  x = xh + xl, w = wh + wl, logits = xh@wh + xh@wl + xl@wh
as three bf16 matmuls per 128-hidden chunk accumulated in one PSUM tile.

Host prepares x in exact DMA consumption order
  [block, partition, hi/lo, chunk, token]
so each input block transfer is one dense 2D copy (16KB contiguous per
partition -> 4KB DMA packets at full HBM rate). Tokens are permuted so
mm-tile j / partition p holds local token p*32+j, making the [4096,2]
outputs contiguous staged DMAs.

Self-contained: hardcodes shapes; imports only concourse (on PYTHONPATH).
"""

import numpy as np

N_TOKENS = 32768
HIDDEN = 2048
NUM_EXPERTS = 64
TOP_K = 2

N_CORES = 8
NP = N_TOKENS // N_CORES  # tokens per core = 4096
KCH = HIDDEN // 128       # hidden chunks of 128 = 16
BLK = 256                 # tokens per DMA block
NBLK = NP // BLK          # 8
TPB = BLK // 128          # matmul tiles per block = 4
NTILE = NP // 128         # 32 mm-tiles per core

_CACHE = {}


def _build_nc():
    import concourse.bacc as bacc
    import concourse.tile as tile
    from concourse import mybir

    f32 = mybir.dt.float32
    bf16 = mybir.dt.bfloat16
    u32 = mybir.dt.uint32
    Alu = mybir.AluOpType
    Act = mybir.ActivationFunctionType

    nc = bacc.Bacc("TRN2", target_bir_lowering=False, debug=False)

    xc = nc.dram_tensor("xc", [NBLK, 128, 2, KCH, BLK], bf16,
                        kind="ExternalInput").ap()
    whl = nc.dram_tensor("whl", [HIDDEN, 2 * NUM_EXPERTS], bf16,
                         kind="ExternalInput").ap()
    w_out = nc.dram_tensor("w_out", [NP, TOP_K], f32, kind="ExternalOutput").ap()
    i_out = nc.dram_tensor("i_out", [NP, TOP_K], u32, kind="ExternalOutput").ap()
    p_out = nc.dram_tensor("p_out", [1, NUM_EXPERTS], f32, kind="ExternalOutput").ap()

    wv = w_out.rearrange("(p j) k -> p j k", p=128)
    iv = i_out.rearrange("(p j) k -> p j k", p=128)

    with tile.TileContext(nc) as tc:
        with tc.tile_pool(name="xpool", bufs=6) as xpool, \
             tc.tile_pool(name="cpool", bufs=1) as cpool, \
             tc.tile_pool(name="lpool", bufs=4) as lpool, \
             tc.tile_pool(name="spool", bufs=6) as spool, \
             tc.tile_pool(name="pspool", bufs=7, space="PSUM") as pspool, \
             tc.tile_pool(name="pfpool", bufs=1, space="PSUM") as pfpool:

            # ---- constants (HWDGE sync ring, issued before the blocks) ----
            w_sb = cpool.tile([128, KCH, 2 * NUM_EXPERTS], bf16)
            nc.sync.dma_start(out=w_sb,
                              in_=whl.rearrange("(K p) e -> p K e", p=128))

            accP = cpool.tile([128, NUM_EXPERTS], f32)
            nc.vector.memset(accP, 0.0)

            ones = cpool.tile([128, 1], f32)
            nc.vector.memset(ones, 1.0)

            # output staging, one tile per quarter so each stores early
            NQ = 4
            QT = NTILE // NQ
            wstage = [cpool.tile([128, QT, TOP_K], f32,
                                 name=f"wstage{h}", tag=f"wstage{h}")
                      for h in range(NQ)]
            istage = [cpool.tile([128, QT, TOP_K], u32,
                                 name=f"istage{h}", tag=f"istage{h}")
                      for h in range(NQ)]

            for b in range(NBLK):
                xblk = xpool.tile([128, 2, KCH, BLK], bf16, tag="xblk")
                if b in (0, NBLK - 1):
                    # split first/last blocks across both HWDGE rings: the
                    # pipeline starts sooner and the final tile's data (the
                    # critical tail) lands earlier
                    h0 = BLK // 2
                    nc.sync.dma_start(out=xblk[:, :, :, 0:h0],
                                      in_=xc[b][:, :, :, 0:h0])
                    nc.scalar.dma_start(out=xblk[:, :, :, h0:BLK],
                                        in_=xc[b][:, :, :, h0:BLK])
                else:
                    nc.sync.dma_start(out=xblk, in_=xc[b])

                for jj in range(TPB):
                    j = b * TPB + jj
                    half, jh = divmod(j, QT)
                    ts = jj * 128
                    # ps[:, :64] = xh@wh + xl@wh ; ps[:, 64:] = xh@wl
                    ps = pspool.tile([128, 2 * NUM_EXPERTS], f32, tag="ps")
                    for k in range(KCH):
                        nc.tensor.matmul(ps, lhsT=xblk[:, 0, k, ts:ts + 128],
                                         rhs=w_sb[:, k, :],
                                         start=(k == 0), stop=False)
                        nc.tensor.matmul(ps[:, 0:NUM_EXPERTS],
                                         lhsT=xblk[:, 1, k, ts:ts + 128],
                                         rhs=w_sb[:, k, 0:NUM_EXPERTS],
                                         start=False, stop=(k == KCH - 1))

                    hi = lpool.tile([128, NUM_EXPERTS], f32, tag="hi")
                    nc.scalar.activation(hi, ps[:, NUM_EXPERTS:2 * NUM_EXPERTS],
                                         Act.Copy)
                    logit = lpool.tile([128, NUM_EXPERTS], f32, tag="logit")
                    nc.vector.tensor_add(logit, ps[:, 0:NUM_EXPERTS], hi)

                    mx = spool.tile([128, 8], f32, tag="mx")
                    nc.vector.max(out=mx, in_=logit)
                    mi = spool.tile([128, 8], u32, tag="mi")
                    nc.vector.max_index(out=mi, in_max=mx, in_values=logit)

                    negm = spool.tile([128, 1], f32, tag="negm")
                    nc.scalar.activation(negm, mx[:, 0:1], Act.Copy, bias=0.0,
                                         scale=-1.0)

                    # full-row softmax numerator + row sum (for probs partials)
                    ex = lpool.tile([128, NUM_EXPERTS], f32, tag="ex")
                    s = spool.tile([128, 1], f32, tag="s")
                    nc.scalar.activation(ex, logit, Act.Exp, bias=negm, scale=1.0,
                                         accum_out=s)

                    # top-2 softmax: [1, exp(m2-m1)] / (1 + exp(m2-m1))
                    pairex = spool.tile([128, TOP_K], f32, tag="pairex")
                    denom = spool.tile([128, 1], f32, tag="denom")
                    nc.scalar.activation(pairex, mx[:, 0:TOP_K], Act.Exp, bias=negm,
                                         scale=1.0, accum_out=denom)
                    rden = spool.tile([128, 1], f32, tag="rden")
                    nc.vector.reciprocal(rden, denom)
                    nc.vector.tensor_scalar_mul(wstage[half][:, jh, :], pairex, rden)

                    rs = spool.tile([128, 1], f32, tag="rs")
                    nc.vector.reciprocal(rs, s)
                    # accP += ex * (1/s)
                    nc.vector.scalar_tensor_tensor(
                        out=accP, in0=ex, scalar=rs, in1=accP,
                        op0=Alu.mult, op1=Alu.add)

                    nc.vector.tensor_copy(istage[half][:, jh, :], mi[:, 0:TOP_K])

                # store each completed quarter on the scalar (ACT) HWDGE
                # ring; the last quarter goes out in two halves, the final
                # one on the (by then idle) sync ring to shorten the tail
                done = b * TPB + TPB
                if done == NTILE - QT // 2:
                    nc.scalar.dma_start(out=wv[:, NTILE - QT:NTILE - QT // 2, :],
                                        in_=wstage[3][:, 0:QT // 2, :])
                    nc.scalar.dma_start(out=iv[:, NTILE - QT:NTILE - QT // 2, :],
                                        in_=istage[3][:, 0:QT // 2, :])
                elif done == NTILE:
                    nc.sync.dma_start(out=wv[:, NTILE - QT // 2:NTILE, :],
                                      in_=wstage[3][:, QT // 2:QT, :])
                    nc.sync.dma_start(out=iv[:, NTILE - QT // 2:NTILE, :],
                                      in_=istage[3][:, QT // 2:QT, :])
                elif done % QT == 0:
                    half = done // QT - 1
                    j0 = half * QT
                    nc.scalar.dma_start(out=wv[:, j0:j0 + QT, :],
                                        in_=wstage[half])
                    nc.scalar.dma_start(out=iv[:, j0:j0 + QT, :],
                                        in_=istage[half])

            # column-sum accP over the 128 partitions: [1, 64] = ones.T @ accP
            pps = pfpool.tile([1, NUM_EXPERTS], f32, tag="pps")
            nc.tensor.matmul(pps, lhsT=ones, rhs=accP, start=True, stop=True)
            psb = cpool.tile([1, NUM_EXPERTS], f32)
            nc.scalar.activation(psb, pps, Act.Copy)
            nc.sync.dma_start(out=p_out, in_=psb)

    nc.compile()
    return nc


def _get_nc():
    if "nc" not in _CACHE:
        _CACHE["nc"] = _build_nc()
    return _CACHE["nc"]


def _prep_core(x_shard):
    """x_shard [NP, HIDDEN] f32 -> [NBLK, 128, 2, KCH, BLK] bf16.

    Element [b, p, hl, k, t] = hi/lo part of x_shard[tok, k*128+p] for
    tok = perm(b*BLK+t) where kernel column j*128+q holds local token
    q*32+j (so mm-tile j / partition q gets token q*32+j).
    """
    import ml_dtypes

    bf = ml_dtypes.bfloat16
    xt = np.ascontiguousarray(x_shard.T)                # [H, NP] cols = tokens
    xt = xt.reshape(HIDDEN, 128, NTILE).transpose(0, 2, 1).reshape(HIDDEN, NP)
    xh = xt.astype(bf)
    xl = (xt - xh.astype(np.float32)).astype(bf)
    a = np.stack([xh, xl], axis=0)                      # [2, H, NP]
    a = a.reshape(2, KCH, 128, NBLK, BLK)
    a = a.transpose(3, 2, 0, 1, 4)                      # [NBLK, 128, 2, KCH, BLK]
    return np.ascontiguousarray(a)


def _prep_w(gate_w):
    import ml_dtypes

    bf = ml_dtypes.bfloat16
    wh = gate_w.astype(bf)
    wl = (gate_w - wh.astype(np.float32)).astype(bf)
    return np.ascontiguousarray(np.concatenate([wh, wl], axis=1))


def _run(x, gate_w, trace=False, trace_cores=None):
    from concourse import bass_utils

    nc = _get_nc()
    x = np.asarray(x, dtype=np.float32)
    gate_w = np.asarray(gate_w, dtype=np.float32)

    whl = _prep_w(gate_w)
    in_maps = [
        {"xc": _prep_core(x[c * NP:(c + 1) * NP]), "whl": whl}
        for c in range(N_CORES)
    ]
    kw = {}
    if trace:
        kw = {"trace": True,
              "trace_cores": trace_cores if trace_cores is not None else [0]}
    res = bass_utils.run_bass_kernel_spmd(nc, in_maps, list(range(N_CORES)), **kw)
    outs = res.results

    w = np.concatenate([outs[c]["w_out"] for c in range(N_CORES)], axis=0)
    idx = np.concatenate([outs[c]["i_out"] for c in range(N_CORES)],
                         axis=0).astype(np.int32)
    psum = np.stack([outs[c]["p_out"][0] for c in range(N_CORES)],
                    axis=0).sum(axis=0, dtype=np.float64)
    counts = np.bincount(idx.reshape(-1), minlength=NUM_EXPERTS).astype(np.float64)
    f = counts / float(N_TOKENS)
    P = psum / float(N_TOKENS)
    loss = np.float32(NUM_EXPERTS * np.sum(f * P))
    return (w.astype(np.float32), idx, loss), res


def kernel(x, gate_w):
    (w, idx, loss), _ = _run(x, gate_w)
    return w, idx, loss
